# revision 8
# baseline (speedup 1.0000x reference)
"""Grouped linear (MoE expert GEMM) on 8 NeuronCores, expert-parallel.

Problem: hidden_states [16384, 2048] f32, weight [8, 2048, 2048] f32,
tokens_per_expert [8] = 2048 each (balanced). Output [16384, 2048] f32 with
out[g*2048+t, o] = sum_i x[g*2048+t, i] * weight[g, o, i].

Sharding: expert-parallel -- core g gets expert g's weight [2048, 2048] and its
2048 routed tokens; each core runs one 2048x2048x2048 GEMM. No collectives.

Per-core kernel, mixed precision to beat the 1-col/cycle PE floor:
- k 0..1535 (12 chunks of 128) in fp16: 1 col/cycle, 216 ns per 512-wide MM.
- k 1536..2047 (2 pairs of 256) in fp8-e4m3 with perf_mode=DoubleRow:
  2 MACs/cell/cycle, ~109 ns per 512-wide MM covering 256 k.
Both sections accumulate into one PSUM group: all operands carry a shared
power-of-2 scale (x*32, w*8192 -> PSUM holds 2^18 * out), removed by a
tensor_scalar_mul(2^-18) in the PSUM->SBUF copy. Host-simulated rel err on
the real data: 1.63e-2 (gate 2e-2); fp8 quantization dominates.

The loop is ordered (tt, km, oi) so each stationary tile is reused for 4
consecutive matmuls (LDWEIGHTS amortized/hidden). DMA trigger order is the
ramp-critical path: x(tt=0) then all W tiles, then x1..x15. Output is bf16,
batched one DMA per token tile; host upcasts.
"""

import numpy as np

G = 8
TPG = 2048  # tokens per expert (= per core)
IN = 2048
OUT = 2048
P = 128
TT = TPG // P  # 16 token tiles of 128
ON = 4  # number of output-column chunks
OW = OUT // ON  # 512
K16 = 12  # fp16 contraction chunks of 128 (k 0..1535)
KP8 = 2  # fp8 DoubleRow pairs of 256 (k 1536..2047)
SX = 32.0  # power-of-2 scale on x (both sections)
SW = 8192.0  # power-of-2 scale on w (both sections)
DEQ = 1.0 / (SX * SW)  # 2^-18

_nc_cache = {}


def _build_nc():
    import concourse.bacc as bacc
    import concourse.mybir as mybir
    import concourse.tile as tile

    if "nc" in _nc_cache:
        return _nc_cache["nc"]

    f32 = mybir.dt.float32
    bf16 = mybir.dt.bfloat16
    fp16 = mybir.dt.float16
    fp8 = mybir.dt.float8e4
    DR = mybir.MatmulPerfMode.DoubleRow

    nc = bacc.Bacc(None, target_bir_lowering=False)

    # x16[p, tt, km, t] = SX * x[tt*128+t, km*128+p]          (k on partitions)
    x16 = nc.dram_tensor("x16", [P, TT, K16, P], fp16, kind="ExternalInput")
    # w16[p, km, o] = SW * w[o, km*128+p]
    w16 = nc.dram_tensor("w16", [P, K16, OUT], fp16, kind="ExternalInput")
    # x8[p, tt, kp, i, t] = q(SX * x[tt*128+t, 1536 + kp*256 + i*128 + p])
    x8 = nc.dram_tensor("x8", [P, TT, KP8, 2, P], fp8, kind="ExternalInput")
    # w8[p, kp, i, o] = q(SW * w[o, 1536 + kp*256 + i*128 + p])
    w8 = nc.dram_tensor("w8", [P, KP8, 2, OUT], fp8, kind="ExternalInput")
    # out[tt, p, o] = C[tt*128+p, o] (bf16; host upcasts)
    out = nc.dram_tensor("out", [TT, P, OUT], bf16, kind="ExternalOutput")

    with tile.TileContext(nc) as tc:
        with (
            tc.tile_pool(name="warm", bufs=1) as warmpool,
            tc.tile_pool(name="xpool", bufs=1) as xpool,
            tc.tile_pool(name="wpool", bufs=1) as wpool,
            tc.tile_pool(name="opool", bufs=2) as opool,
            tc.tile_pool(name="ppool", bufs=8, space="PSUM") as ppool,
        ):
            # HAM warm-up fodder: the PE clock-gate needs ~3.4 us of sustained
            # matmul activity to go 1.2 -> 2.4 GHz. Dummy matmuls on zeroed
            # tiles run while the first input DMAs are still in flight, so
            # the real stream starts at full clock.
            warm_l = warmpool.tile([P, P], fp16, name="warm_l", tag="wl")
            warm_r = warmpool.tile([P, OW], fp16, name="warm_r", tag="wr")
            nc.any.memset(warm_l[:], 0)
            nc.any.memset(warm_r[:], 0)

            x16t = [
                xpool.tile([P, K16, P], fp16, name=f"x16_{i}", tag=f"x16_{i}")
                for i in range(TT)
            ]
            x8t = [
                xpool.tile([P, KP8, 2, P], fp8, name=f"x8_{i}", tag=f"x8_{i}")
                for i in range(TT)
            ]
            w16t = [
                wpool.tile([P, OUT], fp16, name=f"w16_{k}", tag=f"w16_{k}")
                for k in range(K16)
            ]
            w8t = [
                wpool.tile([P, 2, OUT], fp8, name=f"w8_{k}", tag=f"w8_{k}")
                for k in range(KP8)
            ]

            def x16_lhsT(tt, km):
                return x16t[tt][:, km, :]

            # Ramp-critical DMA order: tt0's x, then the full W stream
            # (consumed in this order by tt0's matmuls), then x1..x15
            # which have ~200 us of slack.
            nc.sync.dma_start(out=x16t[0][:], in_=x16[:, 0])
            nc.sync.dma_start(out=x8t[0][:], in_=x8[:, 0])
            for km in range(K16):
                nc.sync.dma_start(out=w16t[km][:], in_=w16[:, km])
            for kp in range(KP8):
                nc.sync.dma_start(out=w8t[kp][:], in_=w8[:, kp])
            for i in range(1, TT):
                nc.sync.dma_start(out=x16t[i][:], in_=x16[:, i])
                nc.sync.dma_start(out=x8t[i][:], in_=x8[:, i])

            for tt in range(TT):
                psums = [
                    ppool.tile([P, OW], f32, name=f"ps{tt}_{oi}", tag="ps")
                    for oi in range(ON)
                ]
                if tt == 0:
                    for _ in range(14):
                        nc.tensor.matmul(
                            out=psums[0][:],
                            lhsT=warm_l[:],
                            rhs=warm_r[:],
                            start=True,
                            stop=True,
                        )
                o_sb = opool.tile([P, OUT], bf16, name=f"o{tt}", tag="o")
                if tt < TT - 1:
                    # (km, oi) order: each stationary tile feeds 4
                    # consecutive matmuls; one accumulation group per oi.
                    for km in range(K16):
                        for oi in range(ON):
                            nc.tensor.matmul(
                                out=psums[oi][:],
                                lhsT=x16_lhsT(tt, km),
                                rhs=w16t[km][:, oi * OW : (oi + 1) * OW],
                                start=(km == 0),
                                stop=False,
                            )
                    for kp in range(KP8):
                        for oi in range(ON):
                            nc.tensor.matmul(
                                out=psums[oi][:],
                                lhsT=x8t[tt][:, kp, :, :],
                                rhs=w8t[kp][:, :, oi * OW : (oi + 1) * OW],
                                start=False,
                                stop=(kp == KP8 - 1),
                                perf_mode=DR,
                            )
                    for oi in range(ON):
                        nc.vector.tensor_scalar_mul(
                            o_sb[:, oi * OW : (oi + 1) * OW], psums[oi][:], DEQ
                        )
                    nc.scalar.dma_start(out=out[tt], in_=o_sb[:])
                else:
                    # Last tile: oi-outer so each oi's copy + output DMA
                    # overlaps the next oi's matmuls, shrinking the tail.
                    for oi in range(ON):
                        for km in range(K16):
                            nc.tensor.matmul(
                                out=psums[oi][:],
                                lhsT=x16_lhsT(tt, km),
                                rhs=w16t[km][:, oi * OW : (oi + 1) * OW],
                                start=(km == 0),
                                stop=False,
                            )
                        for kp in range(KP8):
                            nc.tensor.matmul(
                                out=psums[oi][:],
                                lhsT=x8t[tt][:, kp, :, :],
                                rhs=w8t[kp][:, :, oi * OW : (oi + 1) * OW],
                                start=False,
                                stop=(kp == KP8 - 1),
                                perf_mode=DR,
                            )
                        nc.vector.tensor_scalar_mul(
                            o_sb[:, oi * OW : (oi + 1) * OW], psums[oi][:], DEQ
                        )
                        nc.scalar.dma_start(
                            out=out[tt, :, oi * OW : (oi + 1) * OW],
                            in_=o_sb[:, oi * OW : (oi + 1) * OW],
                        )

    nc.compile()
    _nc_cache["nc"] = nc
    return nc


def _shard_inputs(hidden_states, weight):
    """Host-side quantize + reshuffle into the kernel's DRAM layouts."""
    import ml_dtypes

    fp8 = ml_dtypes.float8_e4m3  # IEEE e4m3, max 240 == TRN FP8_EXP4
    x = np.asarray(hidden_states, dtype=np.float32)
    w = np.asarray(weight, dtype=np.float32)
    k16 = K16 * P  # 1536
    in_maps = []
    for g in range(G):
        xg = x[g * TPG : (g + 1) * TPG]  # [2048, 2048]
        wg = w[g]  # [out, in]
        # fp16 section, k < 1536: [tt, t, km, p] -> [p, tt, km, t]
        x16 = np.ascontiguousarray(
            (xg[:, :k16] * SX)
            .reshape(TT, P, K16, P)
            .transpose(3, 0, 2, 1)
            .astype(np.float16)
        )
        w16 = np.ascontiguousarray(
            (wg[:, :k16] * SW)
            .reshape(OUT, K16, P)
            .transpose(2, 1, 0)
            .astype(np.float16)
        )
        # fp8 section, k >= 1536: [tt, t, kp, i, p] -> [p, tt, kp, i, t]
        x8 = np.ascontiguousarray(
            np.clip(xg[:, k16:] * SX, -240.0, 240.0)
            .reshape(TT, P, KP8, 2, P)
            .transpose(4, 0, 2, 3, 1)
            .astype(fp8)
        )
        w8 = np.ascontiguousarray(
            np.clip(wg[:, k16:] * SW, -240.0, 240.0)
            .reshape(OUT, KP8, 2, P)
            .transpose(3, 1, 2, 0)
            .astype(fp8)
        )
        in_maps.append({"x16": x16, "w16": w16, "x8": x8, "w8": w8})
    return in_maps


def _run(hidden_states, weight, trace=False, tmpdir=None):
    from concourse.bass_utils import run_bass_kernel_spmd

    nc = _build_nc()
    in_maps = _shard_inputs(hidden_states, weight)
    res = run_bass_kernel_spmd(
        nc, in_maps, core_ids=list(range(G)), trace=trace, tmpdir=tmpdir
    )
    outs = [
        np.asarray(res.results[g]["out"]).astype(np.float32).reshape(TPG, OUT)
        for g in range(G)
    ]
    full = np.concatenate(outs, axis=0)
    return full, res


def kernel(hidden_states, weight, tokens_per_expert=None, **_ignored):
    out, _ = _run(hidden_states, weight, trace=False)
    return out
